# revision 20
# baseline (speedup 1.0000x reference)
"""Trainium2 Bass kernel v3: head-group sharding + AllToAll exchange.

Sharding: core c owns batch g = c//4 and head-group j = c%4 (heads
4j..4j+3 = global inner blocks {2j, 2j+1}), over the FULL 2048-query
sequence.  This removes the 4x K/V-projection replication of the
row-sharded v2 (per-core PE floor drops 246us -> 164us): every
projection (Q, K, V, out) is computed exactly once across the quad.

The price is a cross-core exchange of the normalized attention outputs
OT: core c needs all 16 heads (8 ib blocks) but only for its 512 output
rows.  That is exactly an AllToAll: after head-pair p completes, chunk r
of a2a_in_p holds my pair-p OT^T for global rank r's rows; a2a_out_p[j]
is then rank j's pair-p OT for MY rows = global ib block 2*(j%4)+p.
Measured on these cores (mesh algorithm, separate CC/SDMA silicon,
overlaps freely with compute): ~11us trigger->begin + 5-15us launch-skew
wait + ~12us data for the 1MB transfer.  The pair-A A2A hides entirely
under pair B's ~66us of ACT-bound attention; only the pair-B A2A is
(partially) exposed in the tail.

SPMD wrinkle: chunk indices of MY quad (4q..4q+3) are core-dependent,
which a single SPMD program cannot address.  Solution: every core writes
its OT slice to BOTH candidate chunks (r and r+4), and the receiver
combines out[j] * mask[j] + out[j+4] * mask[j+4] with a host-supplied
per-core 0/1 mask (DVE muls; the wrong-quad term is zeroed).

Attention structure per pair: 8 streams (hh in {0,1} x qc in {0..3});
each stream is 8 units of (2 S-matmuls [128 keys, 512 q] + one
1024-wide exp on ACT), O lagging S by 2 units and accumulating over all
16 key blocks in one PSUM bank; V_sb columns 64:128 are ONES so the O
pad rows compute the softmax denominator for free (v2 trick).  ACT is
the attention-phase bottleneck (16.8M exps = ~132us/core), so all
projection work (Q/K/V chains in pair A, out-proj partial chains +
exchange combines in pair B) is interleaved into stream filler slots.
"""

import sys

for _p in ("/opt/trn_rl_repo", "/root/.axon_site/_ro/trn_rl_repo"):
    if _p not in sys.path:
        sys.path.append(_p)

import numpy as np

B = 2
N = 2048
DM = 1024
H = 16
DH = 64
INNER = H * DH  # 1024
NCORES = 8
SCALE = DH ** -0.5

A = DM // 128   # 8 dm blocks
KB = N // 128   # 16 key blocks
QC = N // 512   # 4 query tiles
G8 = [[0, 1, 2, 3, 4, 5, 6, 7]]

_cached = {}


def _build(use_cc=True):
    import contextlib
    import concourse.bacc as bacc
    import concourse.tile as tile
    import concourse.mybir as mybir

    f32 = mybir.dt.float32
    bf16 = mybir.dt.bfloat16
    Exp = mybir.ActivationFunctionType.Exp

    nc = bacc.Bacc("TRN2", target_bir_lowering=False, debug=False,
                   enable_asserts=False)

    xT_d = nc.dram_tensor("xT", [DM, N], bf16, kind="ExternalInput").ap()
    Wq_d = nc.dram_tensor("Wq", [2, 128, A, 128], bf16,
                          kind="ExternalInput").ap()
    Wk_d = nc.dram_tensor("Wk", [2, 128, A, 128], bf16,
                          kind="ExternalInput").ap()
    Wv_d = nc.dram_tensor("Wv", [128, A, 256], bf16,
                          kind="ExternalInput").ap()
    Wo_d = nc.dram_tensor("Wo", [INNER, DM], bf16, kind="ExternalInput").ap()
    # bo/qmask come host-prebroadcast to 128 partitions: a gpsimd
    # stride-0 broadcast DMA costs ~40us of SWDGE descriptor generation,
    # and anything queued behind it on gpsimd (the collective triggers!)
    # waits that long.
    bo_d = nc.dram_tensor("bo", [128, DM], f32, kind="ExternalInput").ap()
    qmask_d = nc.dram_tensor("qmask", [128, 8], bf16,
                             kind="ExternalInput").ap()
    out_d = nc.dram_tensor("out", [512, DM], bf16,
                           kind="ExternalOutput").ap()

    a2a_in = [nc.dram_tensor(f"a2a_in{p}", [8, 128, 512], bf16,
                             kind="Internal").ap() for p in range(2)]
    a2a_out = [nc.dram_tensor(f"a2a_out{p}", [8, 128, 512], bf16,
                              kind="Internal").ap() for p in range(2)]
    warm_in = nc.dram_tensor("warm_in", [8, 128, 16], bf16,
                             kind="Internal").ap()
    warm_out = nc.dram_tensor("warm_out", [8, 128, 16], bf16,
                              kind="Internal").ap()

    xT_r = xT_d.rearrange("(a p) n -> a p n", p=128)
    Wo_r = Wo_d.rearrange("(ib p) d -> ib p d", p=128)

    with tile.TileContext(nc) as tc, \
         nc.allow_low_precision(reason="bf16 matmul pipeline, validated e2e"), \
         contextlib.ExitStack() as ctx:
        persist = ctx.enter_context(tc.tile_pool(name="persist", bufs=1))
        xT_sb = persist.tile([128, A, N], bf16)        # 32 KB/part
        KT_sb = persist.tile([128, 2, N], bf16)        # 8 KB/part
        V_sb = persist.tile([128, KB, 4, 128], bf16)   # 16 KB/part
        QT_z = persist.tile([128, 2, 2, N], bf16)      # 16 KB/part
        OT_sb = persist.tile([128, 2, N], bf16)        # 8 KB/part
        OTW = persist.tile([128, 8, 512], bf16)        # 8 KB/part
        otr = persist.tile([128, 8, 512], bf16)        # 8 KB/part
        Wq_sb = persist.tile([128, 2, A, 128], bf16)
        Wk_sb = persist.tile([128, 2, A, 128], bf16)
        Wv_sb = persist.tile([128, A, 256], bf16)
        Wo_sb = persist.tile([128, 8, DM], bf16)       # 16 KB/part
        bo_sb = persist.tile([128, DM], f32)
        mask_sb = persist.tile([128, 8], bf16)
        pre_sb = persist.tile([128, 2, 4, 512], bf16)  # partial out + bias
        ob_all = persist.tile([128, 2, 4, 512], bf16)  # final out tiles
        onef = persist.tile([128, 1], f32)
        zerof = persist.tile([128, 1], f32)

        ps_chain = ctx.enter_context(
            tc.tile_pool(name="ps_chain", bufs=2, space="PSUM"))
        sp_pool = ctx.enter_context(
            tc.tile_pool(name="ps_sp", bufs=2, space="PSUM"))
        op_pool = ctx.enter_context(
            tc.tile_pool(name="ps_op", bufs=2, space="PSUM"))
        es_pool = ctx.enter_context(tc.tile_pool(name="p_es", bufs=12))
        rc_pool = ctx.enter_context(tc.tile_pool(name="p_rc", bufs=6))
        ob_pool = ctx.enter_context(tc.tile_pool(name="p_ob", bufs=8))

        nc.vector.memset(onef, 1.0)
        nc.vector.memset(zerof, 0.0)
        if not use_cc:  # perf probe only: results are wrong without the A2A
            nc.vector.memset(otr, 0.0)
        nc.vector.tensor_copy(
            out=QT_z[:, :, :, :],
            in_=zerof.unsqueeze(1).unsqueeze(1).to_broadcast([128, 2, 2, N]))
        nc.vector.tensor_copy(
            out=V_sb[:, :, :, 64:128],
            in_=onef.unsqueeze(1).unsqueeze(1).to_broadcast([128, KB, 4, 64]))


        # ---- initial loads: sync gets Wq + first-half x columns, scalar
        # gets Wk/Wv + second-half columns + Wo (descriptor gen ~0.6us per
        # dma_start, serialized per queue)
        # Descriptor generation is ~0.6us PER dma_start, serialized per
        # queue.  Column block 0 (gates the first matmuls) goes as 8
        # small per-a DMAs split across both queues (first blocks land
        # ~2us in); later column blocks merge to one strided DMA each
        # (per-partition runs stay 1KB-contiguous).
        nc.scalar.dma_start(out=Wk_sb[:, 0, :, :], in_=Wk_d[0])
        nc.sync.dma_start(out=Wq_sb[:, 0, :, :], in_=Wq_d[0])
        for a in range(A):
            eng = nc.sync if a % 2 == 0 else nc.scalar
            eng.dma_start(out=xT_sb[:, a, 0:512], in_=xT_r[a, :, 0:512])
        nc.scalar.dma_start(out=Wv_sb, in_=Wv_d)
        for i, c0 in enumerate((512, 1024, 1536)):
            eng = nc.sync if i % 2 == 0 else nc.scalar
            eng.dma_start(
                out=xT_sb[:, :, c0:c0 + 512],
                in_=xT_r[:, :, c0:c0 + 512].rearrange("a p n -> p a n"))
        nc.sync.dma_start(out=Wq_sb[:, 1, :, :], in_=Wq_d[1])
        nc.scalar.dma_start(out=Wk_sb[:, 1, :, :], in_=Wk_d[1])
        nc.sync.dma_start(out=Wo_sb,
                          in_=Wo_r.rearrange("ib p d -> p ib d"))
        # needed only by the exchange tail -- keep behind the hot loads
        nc.sync.dma_start(out=mask_sb, in_=qmask_d)
        nc.scalar.dma_start(out=bo_sb, in_=bo_d)

        # ---------------- PE chain helpers ----------------
        def q_chain(p, qc):
            t = ps_chain.tile([128, 512], f32, tag="chain", name="chain")
            for a in range(A):
                nc.tensor.matmul(out=t, lhsT=Wq_sb[:, p, a, :],
                                 rhs=xT_sb[:, a, qc * 512:(qc + 1) * 512],
                                 start=(a == 0), stop=(a == A - 1))
            nc.vector.tensor_copy(
                out=QT_z[0:64, p, 0, qc * 512:(qc + 1) * 512], in_=t[0:64, :])
            nc.vector.tensor_copy(
                out=QT_z[64:128, p, 1, qc * 512:(qc + 1) * 512],
                in_=t[64:128, :])

        def k_chain(p, kc):
            t = ps_chain.tile([128, 512], f32, tag="chain", name="chain")
            for a in range(A):
                nc.tensor.matmul(out=t, lhsT=Wk_sb[:, p, a, :],
                                 rhs=xT_sb[:, a, kc * 512:(kc + 1) * 512],
                                 start=(a == 0), stop=(a == A - 1))
            nc.vector.tensor_copy(
                out=KT_sb[:, p, kc * 512:(kc + 1) * 512], in_=t)

        def v_chain(kb):
            t = ps_chain.tile([128, 512], f32, tag="chain", name="chain")
            vp = t[:, 0:256]
            for a in range(A):
                nc.tensor.matmul(out=vp,
                                 lhsT=xT_sb[:, a, kb * 128:(kb + 1) * 128],
                                 rhs=Wv_sb[:, a, :],
                                 start=(a == 0), stop=(a == A - 1))
            nc.vector.tensor_copy(
                out=V_sb[:, kb, :, 0:64],
                in_=vp.rearrange("p (h c) -> p h c", h=4))

        def chainA(dc, qb):  # out-proj even ibs {0,2,4,6} + bias
            # op_pool (not ps_chain): sharing the attention streams' op
            # ring WAR-orders these after the last streams' epilogues --
            # Tile otherwise hoists them mid-pair-B where their wait on
            # the exchange stalls the whole PE queue.
            t = op_pool.tile([128, 512], f32, tag="op", name="op")
            for i, ib in enumerate((0, 2, 4, 6)):
                nc.tensor.matmul(out=t,
                                 lhsT=OTW[:, ib, qb * 128:(qb + 1) * 128],
                                 rhs=Wo_sb[:, ib, dc * 512:(dc + 1) * 512],
                                 start=(i == 0), stop=(i == 3))
            nc.vector.tensor_add(pre_sb[:, dc, qb, :], t,
                                 bo_sb[:, dc * 512:(dc + 1) * 512])

        def chainB(dc, qb):  # out-proj odd ibs {1,3,5,7}, add into ob_all
            t = op_pool.tile([128, 512], f32, tag="op", name="op")
            for i, ib in enumerate((1, 3, 5, 7)):
                nc.tensor.matmul(out=t,
                                 lhsT=OTW[:, ib, qb * 128:(qb + 1) * 128],
                                 rhs=Wo_sb[:, ib, dc * 512:(dc + 1) * 512],
                                 start=(i == 0), stop=(i == 3))
            nc.vector.tensor_add(ob_all[:, dc, qb, :], t,
                                 pre_sb[:, dc, qb, :])

        def combine(p, jq):  # OTW[2jq+p] = otr[jq]*m[jq] + otr[jq+4]*m[jq+4]
            # tmps come from the epilogues' den/rcp ring (rc_pool) in two
            # 64-partition halves: the ring WAR keeps these DVE ops from
            # being hoisted ahead of the stream epilogues, where their
            # wait on the exchange would block the vector queue.
            ib = 2 * jq + p
            for h in (0, 64):  # chunks pre-masked on send: add + copy-out
                t1 = rc_pool.tile([64, 512], f32, tag="den", name="den")
                nc.vector.tensor_add(t1, otr[h:h + 64, jq, :],
                                     otr[h:h + 64, jq + 4, :])
                nc.vector.tensor_copy(out=OTW[h:h + 64, ib, :], in_=t1)

        def stage(p, qc):  # chunk qc and qc+4 of pair p's exchange
            for r in (qc, qc + 4):
                # pair-B staging must NOT go on sync: sync is parked on
                # recv#0 until A2A#0 completes, and a queued read of
                # OT_sb from there WAR-blocks every later stream epilogue
                # (DVE) -> PSUM releases -> PE (the recurring ~15-21us
                # mid-pair-B stall).
                eng = (nc.sync if r < 4 else nc.scalar) if p == 0 \
                    else nc.scalar
                # chunks are masked on the SEND side (DVE slack inside the
                # attention pairs; the OT_sb operand keeps the mul ordered
                # after the epilogue) so the receive combine is one add
                # instead of 2 muls + add on the critical path.
                stg = ob_pool.tile([128, 512], bf16, tag="stg", name="stg")
                nc.vector.tensor_mul(
                    stg, OT_sb[:, p, qc * 512:(qc + 1) * 512],
                    mask_sb[:, r:r + 1].to_broadcast([128, 512]))
                eng.dma_start(out=a2a_in[p][r], in_=stg)

        def warm():
            # Tiny dummy A2A queued between the real ones keeps the CC
            # core's processing loop awake -- A2A#1 then starts ~1us
            # after its trigger instead of ~11us.  Source is xT_sb
            # (static after load): an OT_sb read here would WAR-block
            # later epilogues; scalar queue so it fires mid-pair-B.
            nc.scalar.dma_start(
                out=warm_in.rearrange("j p f -> p j f"),
                in_=xT_sb[:, 0, 0:128].rearrange("p (j f) -> p j f", j=8))
            if use_cc:
                nc.gpsimd.collective_compute(
                    "AllToAll", mybir.AluOpType.bypass, replica_groups=G8,
                    ins=[warm_in], outs=[warm_out])

        # ---------------- attention stream ----------------
        def attn_stream(p, hh, qc, filler, drain):
            """S/exp/O for head (local 2p+hh), queries qc*512:+512.

            filler: {slot: [callables]} run after S-unit `slot`;
            drain: callables run during the O drain."""
            lh = 2 * p + hh
            pending = []
            opt = op_pool.tile([128, 512], f32, tag="op", name="op")

            def emit_S(ju):
                es = es_pool.tile([128, 2, 512], bf16, tag="es", name="es")
                sp = sp_pool.tile([128, 2, 512], f32, tag="sp", name="sp")
                for u in range(2):
                    kb = 2 * ju + u
                    nc.tensor.matmul(
                        out=sp[:, u, :],
                        lhsT=KT_sb[:, p, kb * 128:(kb + 1) * 128],
                        rhs=QT_z[:, p, hh, qc * 512:(qc + 1) * 512],
                        start=True, stop=True)
                nc.scalar.activation(out=es, in_=sp, func=Exp, scale=SCALE)
                pending.append((ju, es))

            def emit_O():
                ju, es = pending.pop(0)
                for u in range(2):
                    kb = 2 * ju + u
                    nc.tensor.matmul(out=opt, lhsT=V_sb[:, kb, lh, :],
                                     rhs=es[:, u, :],
                                     start=(kb == 0), stop=(kb == KB - 1))

            for t in range(KB // 2):
                emit_S(t)
                for f in filler.get(t, ()):
                    f()
                if t >= 2:
                    emit_O()
            dq = list(drain)
            while pending:
                if dq:
                    dq.pop(0)()
                emit_O()
            while dq:
                dq.pop(0)()
            # rows 64:128 of opt hold the softmax denominator (ones-columns
            # of V_sb); align partitions for the custom-DVE reciprocal.
            den = rc_pool.tile([64, 512], f32, tag="den", name="den")
            nc.vector.tensor_copy(out=den, in_=opt[64:128, :])
            rcp = rc_pool.tile([64, 512], f32, tag="rcp", name="rcp")
            nc.vector.reciprocal_approx_fast(out=rcp, in_=den)
            nc.vector.tensor_mul(
                OT_sb[hh * 64:(hh + 1) * 64, p, qc * 512:(qc + 1) * 512],
                opt[0:64, :], rcp)

        # ---------------- warmup ----------------
        k_chain(0, 0)
        q_chain(0, 0)
        for kb in range(4):
            v_chain(kb)
        k_chain(0, 1)
        for kb in range(4, 8):
            v_chain(kb)
        k_chain(0, 2)
        k_chain(0, 3)

        # ---------------- pair A ----------------
        mk = lambda f, *a: (lambda: f(*a))
        attn_stream(0, 0, 0,
                    {u: [mk(v_chain, 2 * u + 8), mk(v_chain, 2 * u + 9)]
                     for u in range(4)},
                    [mk(q_chain, 0, 1)])
        attn_stream(0, 0, 1,
                    {0: [mk(q_chain, 0, 2)], 2: [mk(q_chain, 0, 3)],
                     4: [mk(k_chain, 1, 0)]},
                    [mk(k_chain, 1, 1)])
        attn_stream(0, 0, 2,
                    {0: [mk(k_chain, 1, 2)], 2: [mk(k_chain, 1, 3)],
                     4: [mk(q_chain, 1, 0)]},
                    [mk(q_chain, 1, 1)])
        attn_stream(0, 0, 3,
                    {0: [mk(q_chain, 1, 2)], 2: [mk(q_chain, 1, 3)]}, [])
        attn_stream(0, 1, 0, {}, [])
        attn_stream(0, 1, 1, {}, [])
        attn_stream(0, 1, 2, {}, [mk(stage, 0, 0)])
        attn_stream(0, 1, 3, {}, [mk(stage, 0, 1), mk(stage, 0, 2)])
        stage(0, 3)  # chunk 3 needs this stream's own epilogue OT write
        if use_cc:
            nc.gpsimd.collective_compute(
                "AllToAll", mybir.AluOpType.bypass, replica_groups=G8,
                ins=[a2a_in[0]], outs=[a2a_out[0]])
            # recv pair-A exchange; sync stalls on the collective sem here,
            # which is harmless: nothing else is pending on sync in pair B.
            nc.sync.dma_start(out=otr,
                              in_=a2a_out[0].rearrange("j p f -> p j f"))

        # ---------------- pair B ----------------
        attn_stream(1, 0, 0, {}, [])
        attn_stream(1, 0, 1, {}, [])
        attn_stream(1, 0, 2, {}, [])
        attn_stream(1, 0, 3, {}, [])
        attn_stream(1, 1, 0, {}, [])
        attn_stream(1, 1, 1, {}, [warm])
        # combines#0 here: their den/rcp-ring slots land right after
        # (1,1,1)'s epilogue, so they run under the last two ACT-bound
        # streams and OTW is ready the moment pair B ends (recv#0 has
        # been sitting in otr since ~mid-pair-B)
        for jq in range(4):
            combine(0, jq)
        attn_stream(1, 1, 2, {}, [mk(stage, 1, 0)])
        attn_stream(1, 1, 3, {}, [mk(stage, 1, 1), mk(stage, 1, 2)])
        stage(1, 3)
        if use_cc:
            nc.gpsimd.collective_compute(
                "AllToAll", mybir.AluOpType.bypass, replica_groups=G8,
                ins=[a2a_in[1]], outs=[a2a_out[1]])
        # the even-ib out-proj chains run during the A2A#1 flight
        for dc in range(2):
            for qb in range(4):
                chainA(dc, qb)
        if use_cc:
            # per-chunk contiguous recv (no 1024-descriptor strided
            # storm), two queues, each combine pipelined right behind
            # its pair of chunks -- scalar is free of exps by now
            for jq in range(4):
                nc.sync.dma_start(out=otr[:, jq, :], in_=a2a_out[1][jq])
                nc.scalar.dma_start(out=otr[:, jq + 4, :],
                                    in_=a2a_out[1][jq + 4])
                combine(1, jq)
        else:
            for jq in range(4):
                combine(1, jq)
        for i, (dc, qb) in enumerate([(d, q) for d in range(2)
                                      for q in range(4)]):
            chainB(dc, qb)
            eng = nc.sync if i % 2 == 0 else nc.scalar  # scalar is free
            eng.dma_start(                              # of exps by now
                out=out_d[qb * 128:(qb + 1) * 128,
                          dc * 512:(dc + 1) * 512],
                in_=ob_all[:, dc, qb, :])

    nc.compile()
    return nc


import os
def _get_nc():
    if "nc" not in _cached:
        _cached["nc"] = _build(use_cc=os.environ.get("NO_CC") != "1")
    return _cached["nc"]


def kernel(queries, Wq, Wkv, Wo, bo, _trace=False):
    from concourse.bass_utils import run_bass_kernel_spmd
    import ml_dtypes

    bf16 = ml_dtypes.bfloat16
    queries = np.asarray(queries, dtype=np.float32)
    Wq = np.asarray(Wq, dtype=np.float32)
    Wkv = np.asarray(Wkv, dtype=np.float32)

    def pack_blocks(W, cols):  # [DM, C] -> [C//cols, 128, A, cols]
        C = W.shape[1]
        return np.ascontiguousarray(
            W.reshape(A, 128, C // cols, cols).transpose(2, 1, 0, 3)
        ).astype(bf16)

    Wk_full = Wkv[:, :INNER]
    Wv_full = Wkv[:, INNER:]
    Wo_c = np.asarray(Wo, dtype=np.float32).astype(bf16)
    bo = np.asarray(bo, dtype=np.float32)

    nc = _get_nc()

    in_maps = []
    for c in range(NCORES):
        g, j = c // 4, c % 4
        sl = slice(256 * j, 256 * (j + 1))
        xT = np.ascontiguousarray(queries[g].T).astype(bf16)
        qmask = np.zeros(8, dtype=bf16)
        qmask[4 * g:4 * g + 4] = 1.0
        qmask = np.ascontiguousarray(np.broadcast_to(qmask, (128, 8)))
        in_maps.append({
            "xT": xT,
            "Wq": pack_blocks(np.ascontiguousarray(Wq[:, sl]), 128),
            "Wk": pack_blocks(np.ascontiguousarray(Wk_full[:, sl]), 128),
            "Wv": pack_blocks(np.ascontiguousarray(Wv_full[:, sl]), 256)[0],
            "Wo": Wo_c,
            "bo": np.ascontiguousarray(
                np.broadcast_to(bo, (128, DM))).astype(np.float32),
            "qmask": qmask,
        })

    res = run_bass_kernel_spmd(nc, in_maps, list(range(NCORES)),
                               trace=_trace)
    out = np.empty((B, N, DM), dtype=np.float32)
    for c in range(NCORES):
        g, j = c // 4, c % 4
        out[g, 512 * j:512 * (j + 1), :] = np.asarray(
            res.results[c]["out"], dtype=np.float32)
    if _trace:
        return out, res
    return out


if __name__ == "__main__":
    rng = np.random.default_rng(0)
    s = 0.02
    inputs = dict(
        queries=rng.standard_normal((B, N, DM), dtype=np.float32),
        Wq=(rng.standard_normal((DM, INNER), dtype=np.float32) * s),
        Wkv=(rng.standard_normal((DM, 2 * INNER), dtype=np.float32) * s),
        Wo=(rng.standard_normal((INNER, DM), dtype=np.float32) * s),
        bo=(rng.standard_normal((DM,), dtype=np.float32) * s),
    )
    out = kernel(**inputs)
    print("kernel ran, out shape", out.shape)


# revision 21
# speedup vs baseline: 1.0967x; 1.0967x over previous
"""Trainium2 Bass kernel v3: head-group sharding + AllToAll exchange.

Sharding: core c owns batch g = c//4 and head-group j = c%4 (heads
4j..4j+3 = global inner blocks {2j, 2j+1}), over the FULL 2048-query
sequence.  This removes the 4x K/V-projection replication of the
row-sharded v2 (per-core PE floor drops 246us -> 164us): every
projection (Q, K, V, out) is computed exactly once across the quad.

The price is a cross-core exchange of the normalized attention outputs
OT: core c needs all 16 heads (8 ib blocks) but only for its 512 output
rows.  That is exactly an AllToAll: after head-pair p completes, chunk r
of a2a_in_p holds my pair-p OT^T for global rank r's rows; a2a_out_p[j]
is then rank j's pair-p OT for MY rows = global ib block 2*(j%4)+p.
Measured on these cores (mesh algorithm, separate CC/SDMA silicon,
overlaps freely with compute): ~11us trigger->begin + 5-15us launch-skew
wait + ~12us data for the 1MB transfer.  The pair-A A2A hides entirely
under pair B's ~66us of ACT-bound attention; only the pair-B A2A is
(partially) exposed in the tail.

SPMD wrinkle: chunk indices of MY quad (4q..4q+3) are core-dependent,
which a single SPMD program cannot address.  Solution: every core writes
its OT slice to BOTH candidate chunks (r and r+4), and the receiver
combines out[j] * mask[j] + out[j+4] * mask[j+4] with a host-supplied
per-core 0/1 mask (DVE muls; the wrong-quad term is zeroed).

Attention structure per pair: 8 streams (hh in {0,1} x qc in {0..3});
each stream is 8 units of (2 S-matmuls [128 keys, 512 q] + one
1024-wide exp on ACT), O lagging S by 2 units and accumulating over all
16 key blocks in one PSUM bank; V_sb columns 64:128 are ONES so the O
pad rows compute the softmax denominator for free (v2 trick).  ACT is
the attention-phase bottleneck (16.8M exps = ~132us/core), so all
projection work (Q/K/V chains in pair A, out-proj partial chains +
exchange combines in pair B) is interleaved into stream filler slots.
"""

import sys

for _p in ("/opt/trn_rl_repo", "/root/.axon_site/_ro/trn_rl_repo"):
    if _p not in sys.path:
        sys.path.append(_p)

import numpy as np

B = 2
N = 2048
DM = 1024
H = 16
DH = 64
INNER = H * DH  # 1024
NCORES = 8
SCALE = DH ** -0.5

A = DM // 128   # 8 dm blocks
KB = N // 128   # 16 key blocks
QC = N // 512   # 4 query tiles
G8 = [[0, 1, 2, 3, 4, 5, 6, 7]]

_cached = {}


def _build(use_cc=True):
    import contextlib
    import concourse.bacc as bacc
    import concourse.tile as tile
    import concourse.mybir as mybir

    f32 = mybir.dt.float32
    bf16 = mybir.dt.bfloat16
    Exp = mybir.ActivationFunctionType.Exp

    nc = bacc.Bacc("TRN2", target_bir_lowering=False, debug=False,
                   enable_asserts=False)

    xT_d = nc.dram_tensor("xT", [DM, N], bf16, kind="ExternalInput").ap()
    Wq_d = nc.dram_tensor("Wq", [2, 128, A, 128], bf16,
                          kind="ExternalInput").ap()
    Wk_d = nc.dram_tensor("Wk", [2, 128, A, 128], bf16,
                          kind="ExternalInput").ap()
    Wv_d = nc.dram_tensor("Wv", [128, A, 256], bf16,
                          kind="ExternalInput").ap()
    Wo_d = nc.dram_tensor("Wo", [INNER, DM], bf16, kind="ExternalInput").ap()
    # bo/qmask come host-prebroadcast to 128 partitions: a gpsimd
    # stride-0 broadcast DMA costs ~40us of SWDGE descriptor generation,
    # and anything queued behind it on gpsimd (the collective triggers!)
    # waits that long.
    bo_d = nc.dram_tensor("bo", [128, DM], f32, kind="ExternalInput").ap()
    qmask_d = nc.dram_tensor("qmask", [128, 8], bf16,
                             kind="ExternalInput").ap()
    out_d = nc.dram_tensor("out", [512, DM], bf16,
                           kind="ExternalOutput").ap()

    a2a_in = [nc.dram_tensor(f"a2a_in{p}", [8, 128, 512], bf16,
                             kind="Internal").ap() for p in range(2)]
    a2a_out = [nc.dram_tensor(f"a2a_out{p}", [8, 128, 512], bf16,
                              kind="Internal").ap() for p in range(2)]
    warm_in = nc.dram_tensor("warm_in", [8, 128, 16], bf16,
                             kind="Internal").ap()
    warm_out = nc.dram_tensor("warm_out", [8, 128, 16], bf16,
                              kind="Internal").ap()

    xT_r = xT_d.rearrange("(a p) n -> a p n", p=128)
    Wo_r = Wo_d.rearrange("(ib p) d -> ib p d", p=128)

    with tile.TileContext(nc) as tc, \
         nc.allow_low_precision(reason="bf16 matmul pipeline, validated e2e"), \
         contextlib.ExitStack() as ctx:
        persist = ctx.enter_context(tc.tile_pool(name="persist", bufs=1))
        xT_sb = persist.tile([128, A, N], bf16)        # 32 KB/part
        KT_sb = persist.tile([128, 2, N], bf16)        # 8 KB/part
        V_sb = persist.tile([128, KB, 4, 128], bf16)   # 16 KB/part
        QT_z = persist.tile([128, 2, 2, N], bf16)      # 16 KB/part
        OT_sb = persist.tile([128, 2, N], bf16)        # 8 KB/part
        OTW = persist.tile([128, 8, 512], bf16)        # 8 KB/part
        otr = persist.tile([128, 8, 512], bf16)        # 8 KB/part
        Wq_sb = persist.tile([128, 2, A, 128], bf16)
        Wk_sb = persist.tile([128, 2, A, 128], bf16)
        Wv_sb = persist.tile([128, A, 256], bf16)
        Wo_sb = persist.tile([128, 8, DM], bf16)       # 16 KB/part
        bo_sb = persist.tile([128, DM], f32)
        mask_sb = persist.tile([128, 8], bf16)
        pre_sb = persist.tile([128, 2, 4, 512], bf16)  # partial out + bias
        ob_all = persist.tile([128, 2, 4, 512], bf16)  # final out tiles
        onef = persist.tile([128, 1], f32)
        zerof = persist.tile([128, 1], f32)

        ps_chain = ctx.enter_context(
            tc.tile_pool(name="ps_chain", bufs=2, space="PSUM"))
        sp_pool = ctx.enter_context(
            tc.tile_pool(name="ps_sp", bufs=2, space="PSUM"))
        op_pool = ctx.enter_context(
            tc.tile_pool(name="ps_op", bufs=2, space="PSUM"))
        es_pool = ctx.enter_context(tc.tile_pool(name="p_es", bufs=12))
        rc_pool = ctx.enter_context(tc.tile_pool(name="p_rc", bufs=6))
        ob_pool = ctx.enter_context(tc.tile_pool(name="p_ob", bufs=4))

        nc.vector.memset(onef, 1.0)
        nc.vector.memset(zerof, 0.0)
        if not use_cc:  # perf probe only: results are wrong without the A2A
            nc.vector.memset(otr, 0.0)
        nc.vector.tensor_copy(
            out=QT_z[:, :, :, :],
            in_=zerof.unsqueeze(1).unsqueeze(1).to_broadcast([128, 2, 2, N]))
        nc.vector.tensor_copy(
            out=V_sb[:, :, :, 64:128],
            in_=onef.unsqueeze(1).unsqueeze(1).to_broadcast([128, KB, 4, 64]))


        # ---- initial loads: sync gets Wq + first-half x columns, scalar
        # gets Wk/Wv + second-half columns + Wo (descriptor gen ~0.6us per
        # dma_start, serialized per queue)
        # Descriptor generation is ~0.6us PER dma_start, serialized per
        # queue.  Column block 0 (gates the first matmuls) goes as 8
        # small per-a DMAs split across both queues (first blocks land
        # ~2us in); later column blocks merge to one strided DMA each
        # (per-partition runs stay 1KB-contiguous).
        nc.scalar.dma_start(out=Wk_sb[:, 0, :, :], in_=Wk_d[0])
        nc.sync.dma_start(out=Wq_sb[:, 0, :, :], in_=Wq_d[0])
        for a in range(A):
            eng = nc.sync if a % 2 == 0 else nc.scalar
            eng.dma_start(out=xT_sb[:, a, 0:512], in_=xT_r[a, :, 0:512])
        nc.scalar.dma_start(out=Wv_sb, in_=Wv_d)
        for i, c0 in enumerate((512, 1024, 1536)):
            eng = nc.sync if i % 2 == 0 else nc.scalar
            eng.dma_start(
                out=xT_sb[:, :, c0:c0 + 512],
                in_=xT_r[:, :, c0:c0 + 512].rearrange("a p n -> p a n"))
        nc.sync.dma_start(out=Wq_sb[:, 1, :, :], in_=Wq_d[1])
        nc.scalar.dma_start(out=Wk_sb[:, 1, :, :], in_=Wk_d[1])
        nc.sync.dma_start(out=Wo_sb,
                          in_=Wo_r.rearrange("ib p d -> p ib d"))
        # needed only by the exchange tail -- keep behind the hot loads
        nc.sync.dma_start(out=mask_sb, in_=qmask_d)
        nc.scalar.dma_start(out=bo_sb, in_=bo_d)

        # ---------------- PE chain helpers ----------------
        def q_chain(p, qc):
            t = ps_chain.tile([128, 512], f32, tag="chain", name="chain")
            for a in range(A):
                nc.tensor.matmul(out=t, lhsT=Wq_sb[:, p, a, :],
                                 rhs=xT_sb[:, a, qc * 512:(qc + 1) * 512],
                                 start=(a == 0), stop=(a == A - 1))
            nc.vector.tensor_copy(
                out=QT_z[0:64, p, 0, qc * 512:(qc + 1) * 512], in_=t[0:64, :])
            nc.vector.tensor_copy(
                out=QT_z[64:128, p, 1, qc * 512:(qc + 1) * 512],
                in_=t[64:128, :])

        def k_chain(p, kc):
            t = ps_chain.tile([128, 512], f32, tag="chain", name="chain")
            for a in range(A):
                nc.tensor.matmul(out=t, lhsT=Wk_sb[:, p, a, :],
                                 rhs=xT_sb[:, a, kc * 512:(kc + 1) * 512],
                                 start=(a == 0), stop=(a == A - 1))
            nc.vector.tensor_copy(
                out=KT_sb[:, p, kc * 512:(kc + 1) * 512], in_=t)

        def v_chain(kb):
            t = ps_chain.tile([128, 512], f32, tag="chain", name="chain")
            vp = t[:, 0:256]
            for a in range(A):
                nc.tensor.matmul(out=vp,
                                 lhsT=xT_sb[:, a, kb * 128:(kb + 1) * 128],
                                 rhs=Wv_sb[:, a, :],
                                 start=(a == 0), stop=(a == A - 1))
            nc.vector.tensor_copy(
                out=V_sb[:, kb, :, 0:64],
                in_=vp.rearrange("p (h c) -> p h c", h=4))

        def chainA(dc, qb):  # out-proj even ibs {0,2,4,6} + bias
            # op_pool (not ps_chain): sharing the attention streams' op
            # ring WAR-orders these after the last streams' epilogues --
            # Tile otherwise hoists them mid-pair-B where their wait on
            # the exchange stalls the whole PE queue.
            t = op_pool.tile([128, 512], f32, tag="op", name="op")
            for i, ib in enumerate((0, 2, 4, 6)):
                nc.tensor.matmul(out=t,
                                 lhsT=OTW[:, ib, qb * 128:(qb + 1) * 128],
                                 rhs=Wo_sb[:, ib, dc * 512:(dc + 1) * 512],
                                 start=(i == 0), stop=(i == 3))
            nc.vector.tensor_add(pre_sb[:, dc, qb, :], t,
                                 bo_sb[:, dc * 512:(dc + 1) * 512])

        def chainB(dc, qb):  # out-proj odd ibs {1,3,5,7}, add into ob_all
            t = op_pool.tile([128, 512], f32, tag="op", name="op")
            for i, ib in enumerate((1, 3, 5, 7)):
                nc.tensor.matmul(out=t,
                                 lhsT=OTW[:, ib, qb * 128:(qb + 1) * 128],
                                 rhs=Wo_sb[:, ib, dc * 512:(dc + 1) * 512],
                                 start=(i == 0), stop=(i == 3))
            nc.vector.tensor_add(ob_all[:, dc, qb, :], t,
                                 pre_sb[:, dc, qb, :])

        def combine(p, jq):  # OTW[2jq+p] = otr[jq]*m[jq] + otr[jq+4]*m[jq+4]
            # tmps come from the epilogues' den/rcp ring (rc_pool) in two
            # 64-partition halves: the ring WAR keeps these DVE ops from
            # being hoisted ahead of the stream epilogues, where their
            # wait on the exchange would block the vector queue.
            ib = 2 * jq + p
            for h in (0, 64):  # chunks pre-masked on send: add + copy-out
                t1 = rc_pool.tile([64, 512], f32, tag="den", name="den")
                nc.vector.tensor_add(t1, otr[h:h + 64, jq, :],
                                     otr[h:h + 64, jq + 4, :])
                nc.vector.tensor_copy(out=OTW[h:h + 64, ib, :], in_=t1)

        def stage(p, qc):  # chunk qc and qc+4 of pair p's exchange
            for r in (qc, qc + 4):
                # pair-B staging must NOT go on sync: sync is parked on
                # recv#0 until A2A#0 completes, and a queued read of
                # OT_sb from there WAR-blocks every later stream epilogue
                # (DVE) -> PSUM releases -> PE (the recurring ~15-21us
                # mid-pair-B stall).
                eng = (nc.sync if r < 4 else nc.scalar) if p == 0 \
                    else nc.scalar
                # chunks are masked on the SEND side (DVE slack inside the
                # attention pairs; the OT_sb operand keeps the mul ordered
                # after the epilogue) so the receive combine is one add
                # instead of 2 muls + add on the critical path.
                stg = ob_pool.tile([128, 512], bf16, tag="stg", name="stg")
                nc.vector.tensor_mul(
                    stg, OT_sb[:, p, qc * 512:(qc + 1) * 512],
                    mask_sb[:, r:r + 1].to_broadcast([128, 512]))
                eng.dma_start(out=a2a_in[p][r], in_=stg)

        def warm():
            # Tiny dummy A2A queued between the real ones keeps the CC
            # core's processing loop awake -- A2A#1 then starts ~1us
            # after its trigger instead of ~11us.  Source is xT_sb
            # (static after load): an OT_sb read here would WAR-block
            # later epilogues; scalar queue so it fires mid-pair-B.
            nc.scalar.dma_start(
                out=warm_in.rearrange("j p f -> p j f"),
                in_=xT_sb[:, 0, 0:128].rearrange("p (j f) -> p j f", j=8))
            if use_cc:
                nc.gpsimd.collective_compute(
                    "AllToAll", mybir.AluOpType.bypass, replica_groups=G8,
                    ins=[warm_in], outs=[warm_out])

        # ---------------- attention stream ----------------
        def attn_stream(p, hh, qc, filler, drain):
            """S/exp/O for head (local 2p+hh), queries qc*512:+512.

            filler: {slot: [callables]} run after S-unit `slot`;
            drain: callables run during the O drain."""
            lh = 2 * p + hh
            pending = []
            opt = op_pool.tile([128, 512], f32, tag="op", name="op")

            def emit_S(ju):
                es = es_pool.tile([128, 2, 512], bf16, tag="es", name="es")
                sp = sp_pool.tile([128, 2, 512], f32, tag="sp", name="sp")
                for u in range(2):
                    kb = 2 * ju + u
                    nc.tensor.matmul(
                        out=sp[:, u, :],
                        lhsT=KT_sb[:, p, kb * 128:(kb + 1) * 128],
                        rhs=QT_z[:, p, hh, qc * 512:(qc + 1) * 512],
                        start=True, stop=True)
                nc.scalar.activation(out=es, in_=sp, func=Exp, scale=SCALE)
                pending.append((ju, es))

            def emit_O():
                ju, es = pending.pop(0)
                for u in range(2):
                    kb = 2 * ju + u
                    nc.tensor.matmul(out=opt, lhsT=V_sb[:, kb, lh, :],
                                     rhs=es[:, u, :],
                                     start=(kb == 0), stop=(kb == KB - 1))

            for t in range(KB // 2):
                emit_S(t)
                for f in filler.get(t, ()):
                    f()
                if t >= 2:
                    emit_O()
            dq = list(drain)
            while pending:
                if dq:
                    dq.pop(0)()
                emit_O()
            while dq:
                dq.pop(0)()
            # rows 64:128 of opt hold the softmax denominator (ones-columns
            # of V_sb); align partitions for the custom-DVE reciprocal.
            den = rc_pool.tile([64, 512], f32, tag="den", name="den")
            nc.vector.tensor_copy(out=den, in_=opt[64:128, :])
            rcp = rc_pool.tile([64, 512], f32, tag="rcp", name="rcp")
            nc.vector.reciprocal_approx_fast(out=rcp, in_=den)
            nc.vector.tensor_mul(
                OT_sb[hh * 64:(hh + 1) * 64, p, qc * 512:(qc + 1) * 512],
                opt[0:64, :], rcp)

        # ---------------- warmup ----------------
        k_chain(0, 0)
        q_chain(0, 0)
        for kb in range(4):
            v_chain(kb)
        k_chain(0, 1)
        for kb in range(4, 8):
            v_chain(kb)
        k_chain(0, 2)
        k_chain(0, 3)

        # ---------------- pair A ----------------
        mk = lambda f, *a: (lambda: f(*a))
        attn_stream(0, 0, 0,
                    {u: [mk(v_chain, 2 * u + 8), mk(v_chain, 2 * u + 9)]
                     for u in range(4)},
                    [mk(q_chain, 0, 1)])
        attn_stream(0, 0, 1,
                    {0: [mk(q_chain, 0, 2)], 2: [mk(q_chain, 0, 3)],
                     4: [mk(k_chain, 1, 0)]},
                    [mk(k_chain, 1, 1)])
        attn_stream(0, 0, 2,
                    {0: [mk(k_chain, 1, 2)], 2: [mk(k_chain, 1, 3)],
                     4: [mk(q_chain, 1, 0)]},
                    [mk(q_chain, 1, 1)])
        attn_stream(0, 0, 3,
                    {0: [mk(q_chain, 1, 2)], 2: [mk(q_chain, 1, 3)]}, [])
        attn_stream(0, 1, 0, {}, [])
        attn_stream(0, 1, 1, {}, [])
        attn_stream(0, 1, 2, {}, [mk(stage, 0, 0)])
        attn_stream(0, 1, 3, {}, [mk(stage, 0, 1), mk(stage, 0, 2)])
        stage(0, 3)  # chunk 3 needs this stream's own epilogue OT write
        if use_cc:
            nc.gpsimd.collective_compute(
                "AllToAll", mybir.AluOpType.bypass, replica_groups=G8,
                ins=[a2a_in[0]], outs=[a2a_out[0]])
            # recv pair-A exchange; sync stalls on the collective sem here,
            # which is harmless: nothing else is pending on sync in pair B.
            nc.sync.dma_start(out=otr,
                              in_=a2a_out[0].rearrange("j p f -> p j f"))

        # ---------------- pair B ----------------
        attn_stream(1, 0, 0, {}, [])
        attn_stream(1, 0, 1, {}, [])
        attn_stream(1, 0, 2, {}, [])
        attn_stream(1, 0, 3, {}, [])
        attn_stream(1, 1, 0, {}, [])
        attn_stream(1, 1, 1, {}, [warm])
        # combines#0 here: their den/rcp-ring slots land right after
        # (1,1,1)'s epilogue, so they run under the last two ACT-bound
        # streams and OTW is ready the moment pair B ends (recv#0 has
        # been sitting in otr since ~mid-pair-B)
        for jq in range(4):
            combine(0, jq)
        attn_stream(1, 1, 2, {}, [mk(stage, 1, 0)])
        attn_stream(1, 1, 3, {}, [mk(stage, 1, 1), mk(stage, 1, 2)])
        stage(1, 3)
        if use_cc:
            nc.gpsimd.collective_compute(
                "AllToAll", mybir.AluOpType.bypass, replica_groups=G8,
                ins=[a2a_in[1]], outs=[a2a_out[1]])
        # the even-ib out-proj chains run during the A2A#1 flight
        for dc in range(2):
            for qb in range(4):
                chainA(dc, qb)
        if use_cc:
            # per-chunk contiguous recv (no 1024-descriptor strided
            # storm), two queues, each combine pipelined right behind
            # its pair of chunks -- scalar is free of exps by now
            for jq in range(4):
                nc.sync.dma_start(out=otr[:, jq, :], in_=a2a_out[1][jq])
                nc.scalar.dma_start(out=otr[:, jq + 4, :],
                                    in_=a2a_out[1][jq + 4])
                combine(1, jq)
        else:
            for jq in range(4):
                combine(1, jq)
        for i, (dc, qb) in enumerate([(d, q) for d in range(2)
                                      for q in range(4)]):
            chainB(dc, qb)
            eng = nc.sync if i % 2 == 0 else nc.scalar  # scalar is free
            eng.dma_start(                              # of exps by now
                out=out_d[qb * 128:(qb + 1) * 128,
                          dc * 512:(dc + 1) * 512],
                in_=ob_all[:, dc, qb, :])

    nc.compile()
    return nc


import os
def _get_nc():
    if "nc" not in _cached:
        _cached["nc"] = _build(use_cc=os.environ.get("NO_CC") != "1")
    return _cached["nc"]


def kernel(queries, Wq, Wkv, Wo, bo, _trace=False):
    from concourse.bass_utils import run_bass_kernel_spmd
    import ml_dtypes

    bf16 = ml_dtypes.bfloat16
    queries = np.asarray(queries, dtype=np.float32)
    Wq = np.asarray(Wq, dtype=np.float32)
    Wkv = np.asarray(Wkv, dtype=np.float32)

    def pack_blocks(W, cols):  # [DM, C] -> [C//cols, 128, A, cols]
        C = W.shape[1]
        return np.ascontiguousarray(
            W.reshape(A, 128, C // cols, cols).transpose(2, 1, 0, 3)
        ).astype(bf16)

    Wk_full = Wkv[:, :INNER]
    Wv_full = Wkv[:, INNER:]
    Wo_c = np.asarray(Wo, dtype=np.float32).astype(bf16)
    bo = np.asarray(bo, dtype=np.float32)

    nc = _get_nc()

    in_maps = []
    for c in range(NCORES):
        g, j = c // 4, c % 4
        sl = slice(256 * j, 256 * (j + 1))
        xT = np.ascontiguousarray(queries[g].T).astype(bf16)
        qmask = np.zeros(8, dtype=bf16)
        qmask[4 * g:4 * g + 4] = 1.0
        qmask = np.ascontiguousarray(np.broadcast_to(qmask, (128, 8)))
        in_maps.append({
            "xT": xT,
            "Wq": pack_blocks(np.ascontiguousarray(Wq[:, sl]), 128),
            "Wk": pack_blocks(np.ascontiguousarray(Wk_full[:, sl]), 128),
            "Wv": pack_blocks(np.ascontiguousarray(Wv_full[:, sl]), 256)[0],
            "Wo": Wo_c,
            "bo": np.ascontiguousarray(
                np.broadcast_to(bo, (128, DM))).astype(np.float32),
            "qmask": qmask,
        })

    res = run_bass_kernel_spmd(nc, in_maps, list(range(NCORES)),
                               trace=_trace)
    out = np.empty((B, N, DM), dtype=np.float32)
    for c in range(NCORES):
        g, j = c // 4, c % 4
        out[g, 512 * j:512 * (j + 1), :] = np.asarray(
            res.results[c]["out"], dtype=np.float32)
    if _trace:
        return out, res
    return out


if __name__ == "__main__":
    rng = np.random.default_rng(0)
    s = 0.02
    inputs = dict(
        queries=rng.standard_normal((B, N, DM), dtype=np.float32),
        Wq=(rng.standard_normal((DM, INNER), dtype=np.float32) * s),
        Wkv=(rng.standard_normal((DM, 2 * INNER), dtype=np.float32) * s),
        Wo=(rng.standard_normal((INNER, DM), dtype=np.float32) * s),
        bo=(rng.standard_normal((DM,), dtype=np.float32) * s),
    )
    out = kernel(**inputs)
    print("kernel ran, out shape", out.shape)
